# revision 72
# baseline (speedup 1.0000x reference)
"""Trainium2 Bass kernel for nn_CrossAttention_14207751815513.

Single-query cross-attention:
    q = x1 @ Wq.T                 (one query per head)
    k = x2 @ Wk.T ; v = x2 @ Wv.T
    attn_h = softmax(q_h . k_h / sqrt(128))
    out = concat_h(attn_h @ v_h) @ Wo.T + bo

Because there is exactly ONE query, the K and V projections collapse
algebraically (associativity):
    scores_h = x2 @ r_h,  r_h = Wk_h.T q_h / sqrt(128)   -- no k materialization
    out_h    = Wv_h @ (x2.T p_h) / l_h                   -- no v materialization
with p = exp(scores) (logits are small, |s| < ~6, so no max subtraction
is needed) and l_h = sum_s p_h[s].

Sharding: the sequence dim (16384) is split across the 8 NeuronCores
(2048 rows each).  Every quantity that is O(1) in the sequence length
(q, R = [r_1..r_16], the per-head Wv matvec, Wo + bias) lives in the
host-side shard-prep / gather-merge glue; the O(S*C) work runs on
device.

v2 changes vs the bf16 baseline (78us -> ~44us):
  * x2 streams (both layouts) are fp8e4 (e4m3): halves HBM->SBUF DMA
    bytes.  R (scaled x64 on the host into e4m3/bf16-friendly range;
    Exp applies scale=1/64) and PT stay bf16 -- the PE accepts mixed
    bf16-stationary x fp8-moving, and this keeps rel-err ~1.26e-2.
  * 4-way PE column tiling: all the big matmuls have only 16 output
    rows, so four run concurrently in distinct 32-column groups of the
    128x128 array (tile_position (0, 32g); one PSUM bank per group --
    interleaved accumulation groups sharing a bank corrupt has_written
    state).  Effective moving-operand ingest ~4 cols/cycle.
      S  : col-group g accumulates scores for s-block g (512 cols)
           over 16 c-chunks; all four into their own PSUM banks.
      exp: 4 ACT ops [16@32g, 512] -> P bf16, partition-aligned, with
           accum_out producing the per-head row sums l for free.
      tr : P[32g:32g+16, 128j:..] -> PT [128, 16] bf16 via PE
           transposes (identity replicated at partition bases 32g),
           then DVE tensor_scalar_mul x0.25 casts PT to... bf16 PT
           scaled by 1/4 so the later fp8-free path stays in range
           (exp(max ~6)=376 > fp8e4 max normal 240 was the nan source
           when PT was fp8; with bf16 PT the x0.25 simply keeps l and
           t consistent: host divides l by 4).
      T  : col-group g accumulates t for c-block g (512 cols) over 16
           s-chunks into its own PSUM bank; out[32g+h, j]=t[h,512g+j].
  * DMA schedule: 13 input stream DMAs in expected arrival order,
    ring-balanced, fine pieces early and where consumption trails;
    output as two small SWDGE (gpsimd) bf16 DMAs so no HW-DGE
    sem-slot-recycle wait ever stacks on a RAW wait.

Outputs per core: tt [128, 513] bf16 (4 partition groups of 16 heads x
512 c-cols, plus P-sums l in the last column of each group).  Host
merge: sum partials over cores, normalize by l/4, apply Wv per head,
then Wo + bo.

Sync-wait note: this backend disables DynamicDMA, so every HW-DGE DMA
lowers to a pseudo-direct DMA that supports at most ONE semaphore wait
("Too many sync wait commands" in walrus codegen otherwise).  The
program is therefore structured so no DMACopy ever needs two waits:
  - every streamed tile is a fresh buffer (unique pool tag, no reuse)
    so stream DMAs carry no WAR/WAW waits;
  - the program issues exactly 8 DMAs total (the 8 HW-DGE semaphore
    slots are assigned globally round-robin across both rings), so no
    DMA ever carries a slot-recycle wait on top of a RAW wait;
  - the output DMA's producers are all on the scalar engine;
  - the end-of-context Drain gets a sem wait for every proc the SP
    engine hasn't directly observed, so an epilogue of single-dep SP
    nops makes SP observe each DMA and each engine's last instruction.
"""

import sys

for _p in ("/root/.axon_site/_ro/trn_rl_repo", "/opt/trn_rl_repo"):
    if _p not in sys.path:
        sys.path.append(_p)

import numpy as np
import ml_dtypes

import concourse.bass as bass
import concourse.tile as tile
from concourse import mybir
from concourse.bass_utils import run_bass_kernel_spmd
from concourse.tile_rust import add_dep_helper

NCORES = 8
S_FULL = 16384
C = 2048           # input feature dim (both x1 and x2)
H = 16             # heads
J = 128            # head dim (K_DIM == V_DIM == 128)
HJ = H * J         # 2048
ODIM = 512
S_LOC = S_FULL // NCORES   # 2048 sequence rows per core

BF = mybir.dt.bfloat16
F32 = mybir.dt.float32
F8 = mybir.dt.float8e4
INV_SQRT_K = 1.0 / float(np.sqrt(128.0))
RSCALE = 64.0      # host multiplies R by this; Exp activation divides

NB = 512                    # PSUM bank free-dim (f32 columns)
CH = C // 128               # 16 chunks of 128 along any 2048 dim
NG = 4                      # column-tile groups

_F8_NP = ml_dtypes.float8_e4m3fn


PSCALE = 1.0    # PT is bf16 (no fp8 range concern); l and t consistent as-is


def _build_program() -> bass.Bass:
    nc = bass.Bass()
    # x2t/x2n are packed partition-major on the host ([p, chunk, col]) so a
    # multi-chunk stream DMA folds to ONE contiguous descriptor per partition.
    t_in = {
        "rsb": nc.dram_tensor("rsb", [J, CH, H], BF, kind="ExternalInput"),
        "x2t": nc.dram_tensor("x2t", [J, CH, S_LOC], F8, kind="ExternalInput"),
        "x2n": nc.dram_tensor("x2n", [J, CH, C], F8, kind="ExternalInput"),
    }
    t_out = {
        "tt": nc.dram_tensor("tt", [J, NB + 1], BF, kind="ExternalOutput"),
    }

    rsb_d = t_in["rsb"][:, :, :]
    x2t_v = t_in["x2t"][:, :, :]
    x2n_v = t_in["x2n"][:, :, :]
    tt_out = t_out["tt"][:, :]

    # Stream DMAs: (ring, first_chunk, n_chunks) in expected ARRIVAL order
    # (rings drain FIFO and run concurrently; PSUM accumulation is order-free,
    # so the matmul program order follows this schedule).  Input DMAs carry no
    # RAW wait (fresh buffers), so sem-slot recycling beyond 8 DMAs costs each
    # at most its one allowed wait.  Ring bytes are balanced; early pieces are
    # small to start compute sooner.
    PT_SCHED = [(0, 0, 1), (1, 1, 3), (0, 4, 3), (1, 7, 3),
                (0, 10, 3), (1, 13, 3)]
    PN_SCHED = [(0, 0, 1), (0, 1, 2), (1, 3, 3), (0, 6, 3),
                (1, 9, 3), (0, 12, 3), (1, 15, 1)]

    with tile.TileContext(nc) as tc:
        with (
            tc.tile_pool(name="singles", bufs=1) as singles,
            tc.tile_pool(name="sa", bufs=1) as sa,
            tc.tile_pool(name="sb", bufs=1) as sbp,
            tc.tile_pool(name="psbig", bufs=8, space="PSUM") as psbig,
        ):
            ep_targets = []     # representative instrs for the tail funnel
            stream_dmas = []    # input-stream DMAs: funneled EARLY (overlapped)

            # ---- issue every stream DMA up front (both rings) --------------
            Rsb = singles.tile([J, CH, H], BF)
            stream_dmas.append(nc.scalar.dma_start(out=Rsb, in_=rsb_d))

            s_tiles = []        # (first_cc, n_cc, tile)
            for ci, (ring, a, k) in enumerate(PT_SCHED):
                eng = nc.sync if ring == 0 else nc.scalar
                xt = sa.tile([128, k, S_LOC], F8, tag=f"sa_{ci}")
                stream_dmas.append(eng.dma_start(out=xt, in_=x2t_v[:, a : a + k, :]))
                s_tiles.append((a, k, xt))
            t_tiles = []
            for ci, (ring, a, k) in enumerate(PN_SCHED):
                eng = nc.sync if ring == 0 else nc.scalar
                xn = sbp.tile([128, k, C], F8, tag=f"sb_{ci}")
                stream_dmas.append(eng.dma_start(out=xn, in_=x2n_v[:, a : a + k, :]))
                t_tiles.append((a, k, xn))

            # Early drain-funnel for stream DMAs: sync nops that fire as each
            # piece lands (hidden under the stream); the end-of-context Drain
            # itself cannot carry a wait per DMA.
            for t in stream_dmas:
                n = nc.sync.nop(nofuse=True, hint="dep")
                add_dep_helper(n.ins, t.ins, reason="drain-funnel-early")



            # ---- phase S: scores[h, 512g+j] = sum_c R[c, h] x2t[c, 512g+j] -
            # col-group g owns s-block g.  One PSUM bank per group: four
            # interleaved accumulation groups sharing a bank corrupt
            # has_written state (stale-PSUM inf observed on HW).
            ps_s = [
                psbig.tile([J, NB], F32, tag="big", name=f"ps_s{g}")
                for g in range(NG)
            ]
            n_done = 0
            for pi, (a, k, xt) in enumerate(s_tiles):
                for gg in range(k):
                    cc = a + gg
                    n_done += 1
                    for g in range(NG):
                        nc.tensor.matmul(
                            ps_s[g][32 * g : 32 * g + H, :],
                            lhsT=Rsb[:, cc, :],
                            rhs=xt[:, gg, g * NB : (g + 1) * NB],
                            start=(n_done == 1),
                            stop=(n_done == CH),
                            tile_position=(0, 32 * g),
                        )


            # ---- exp (scale un-does RSCALE; |logit| < ~6) ------------------
            Psb = singles.tile([J, NB], BF)
            lacc = singles.tile([J, 1], F32)
            i_act = None
            for g in range(NG):
                # accum_out gives the per-head row sums (l) for free -- the
                # partial sums per s-block land in the same partition group.
                i_act = nc.scalar.activation(
                    out=Psb[32 * g : 32 * g + H, :],
                    in_=ps_s[g][32 * g : 32 * g + H, :],
                    func=mybir.ActivationFunctionType.Exp,
                    scale=1.0 / RSCALE,
                    accum_out=lacc[32 * g : 32 * g + H, :],
                )

            # ---- transpose P -> PT [128, 16 schunk, 32(h pad)] bf16 --------
            # DVE blockwise 32x32 transposes straight from the exp output
            # (SBUF->SBUF, off the PE queue): block b of group g covers
            # s_local = 32b+i, i.e. chunk sc = 4g + b//4, quarter m = b%4.
            # Rows 32g+16:32g+32 of Psb are unwritten pad; they land in PT
            # columns 16:32, which the T matmuls never read.
            PT = singles.tile([J, CH, 32], BF)
            for g in range(NG):
                for b in range(16):
                    sc, m = 4 * g + b // 4, b % 4
                    nc.vector.transpose(
                        out=PT[32 * m : 32 * m + 32, sc, :],
                        in_=Psb[32 * g : 32 * g + 32, 32 * b : 32 * b + 32],
                    )

            # l columns of the output: copied early (deps ready at exp time),
            # hidden under the T-phase stream.
            tt_sb = singles.tile([J, NB + 1], BF)
            i_copies = []
            for g in range(NG):
                i_copies.append(
                    nc.vector.tensor_copy(
                        out=tt_sb[32 * g : 32 * g + H, NB : NB + 1],
                        in_=lacc[32 * g : 32 * g + H, :],
                    )
                )

            # ---- phase T: t[h, 512g+j] = sum_s PT[s, h] x2n[s, 512g+j] -----
            ps_t = [
                psbig.tile([J, NB], F32, tag="big", name=f"ps_t{g}")
                for g in range(NG)
            ]
            i_pe = None
            n_done = 0
            for a, k, xn in t_tiles:
                for gg in range(k):
                    sc = a + gg
                    n_done += 1
                    for g in range(NG):
                        i_pe = nc.tensor.matmul(
                            ps_t[g][32 * g : 32 * g + H, :],
                            lhsT=PT[:, sc, :H],
                            rhs=xn[:, gg, g * NB : (g + 1) * NB],
                            start=(n_done == 1),
                            stop=(n_done == CH),
                            tile_position=(0, 32 * g),
                        )
            # all output copies on DVE (idle at this point) so each output DMA
            # carries exactly one RAW wait; two SWDGE DMAs let the first
            # half's issue/transfer/receipt overlap the second half's copies.
            for g in range(NG):
                i_copies.append(
                    nc.vector.tensor_copy(
                        out=tt_sb[32 * g : 32 * g + H, :NB],
                        in_=ps_t[g][32 * g : 32 * g + H, :],
                    )
                )
            i_out = nc.gpsimd.dma_start(out=tt_out[:48, :], in_=tt_sb[:48, :])
            i_out2 = nc.gpsimd.dma_start(out=tt_out[64:112, :], in_=tt_sb[64:112, :])

            # ---- drain-funnel epilogue (see sync-wait note above): one nop
            # per un-observed proc -- each engine's LAST instruction plus the
            # output DMAs (stream DMAs were funneled early).
            ep_targets += [i_act, i_copies[-1], i_pe, i_out, i_out2]
            for t in ep_targets:
                n = nc.sync.nop(nofuse=True, hint="dep")
                add_dep_helper(n.ins, t.ins, reason="drain-funnel")

    return nc


_NC_CACHE = None


def _get_nc() -> bass.Bass:
    global _NC_CACHE
    if _NC_CACHE is None:
        _NC_CACHE = _build_program()
    return _NC_CACHE


def _prep_in_maps(x1, x2, Wq, Wk):
    x1 = np.asarray(x1, np.float32)
    x2 = np.asarray(x2, np.float32)
    Wq = np.asarray(Wq, np.float32)
    Wk = np.asarray(Wk, np.float32)

    # R[c, h] = sum_j Wk[h*128+j, c] q[h*128+j] / sqrt(128),  scaled by RSCALE
    q = (Wq @ x1) * (INV_SQRT_K * RSCALE)                       # [2048]
    R = np.einsum("hj,hjc->ch", q.reshape(H, J), Wk.reshape(H, J, C))
    rsb = np.ascontiguousarray(
        R.reshape(CH, 128, H).transpose(1, 0, 2)
    ).astype(ml_dtypes.bfloat16)                                # [128, 16, 16]

    in_maps = []
    for c in range(NCORES):
        shard = x2[c * S_LOC : (c + 1) * S_LOC]                 # [2048, 2048]
        # packed partition-major: x2t[p, cc, s] = shard.T[cc*128+p, s],
        #                         x2n[p, sc, c] = shard[sc*128+p, c]
        x2t_c = np.ascontiguousarray(
            shard.T.reshape(CH, 128, S_LOC).transpose(1, 0, 2)
        ).astype(_F8_NP)
        x2n_c = np.ascontiguousarray(
            shard.reshape(CH, 128, C).transpose(1, 0, 2)
        ).astype(_F8_NP)
        in_maps.append({"rsb": rsb, "x2t": x2t_c, "x2n": x2n_c})
    return in_maps


def _merge(results, Wv, Wo, bo):
    Wv = np.asarray(Wv, np.float32)
    Wo = np.asarray(Wo, np.float32)
    bo = np.asarray(bo, np.float32)
    t_tot = np.zeros((H, C), np.float64)
    l_tot = np.zeros(H, np.float64)
    for r in results:
        tt = r["tt"].astype(np.float64)                         # [128, 513]
        for g in range(NG):
            t_tot[:, g * NB : (g + 1) * NB] += tt[32 * g : 32 * g + H, :NB]
            l_tot += tt[32 * g : 32 * g + H, NB] * PSCALE
    tn = t_tot / l_tot[:, None]                                 # [16, 2048]
    u = np.einsum("hc,hjc->hj", tn, Wv.astype(np.float64).reshape(H, J, C))
    out = u.reshape(HJ) @ Wo.T.astype(np.float64) + bo.astype(np.float64)
    return out.astype(np.float32).reshape(1, ODIM)


def kernel(x1, x2, Wq, Wk, Wv, Wo, bo):
    nc = _get_nc()
    in_maps = _prep_in_maps(x1, x2, Wq, Wk)
    res = run_bass_kernel_spmd(nc, in_maps, list(range(NCORES)))
    return _merge(res.results, Wv, Wo, bo)


def run_traced(x1, x2, Wq, Wk, Wv, Wo, bo, **trace_kwargs):
    """Like kernel() but returns (output, BassKernelResults) with NTFF trace."""
    nc = _get_nc()
    in_maps = _prep_in_maps(x1, x2, Wq, Wk)
    res = run_bass_kernel_spmd(
        nc, in_maps, list(range(NCORES)), trace=True, **trace_kwargs
    )
    return _merge(res.results, Wv, Wo, bo), res


# revision 75
# speedup vs baseline: 1.2611x; 1.2611x over previous
"""Trainium2 Bass kernel for nn_CrossAttention_14207751815513.

Single-query cross-attention:
    q = x1 @ Wq.T                 (one query per head)
    k = x2 @ Wk.T ; v = x2 @ Wv.T
    attn_h = softmax(q_h . k_h / sqrt(128))
    out = concat_h(attn_h @ v_h) @ Wo.T + bo

Because there is exactly ONE query, the K and V projections collapse
algebraically (associativity):
    scores_h = x2 @ r_h,  r_h = Wk_h.T q_h / sqrt(128)   -- no k materialization
    out_h    = Wv_h @ (x2.T p_h) / l_h                   -- no v materialization
with p = exp(scores) (logits are small, |s| < ~6, so no max subtraction
is needed) and l_h = sum_s p_h[s].

Sharding: the sequence dim (16384) is split across the 8 NeuronCores
(2048 rows each).  Every quantity that is O(1) in the sequence length
(q, R = [r_1..r_16], the per-head Wv matvec, Wo + bias) lives in the
host-side shard-prep / gather-merge glue; the O(S*C) work runs on
device.

v2 changes vs the bf16 baseline (78us -> ~44us):
  * x2 streams (both layouts) are fp8e4 (e4m3): halves HBM->SBUF DMA
    bytes.  R (scaled x64 on the host into e4m3/bf16-friendly range;
    Exp applies scale=1/64) and PT stay bf16 -- the PE accepts mixed
    bf16-stationary x fp8-moving, and this keeps rel-err ~1.26e-2.
  * 4-way PE column tiling: all the big matmuls have only 16 output
    rows, so four run concurrently in distinct 32-column groups of the
    128x128 array (tile_position (0, 32g); one PSUM bank per group --
    interleaved accumulation groups sharing a bank corrupt has_written
    state).  Effective moving-operand ingest ~4 cols/cycle.
      S  : col-group g accumulates scores for s-block g (512 cols)
           over 16 c-chunks; all four into their own PSUM banks.
      exp: 4 ACT ops [16@32g, 512] -> P bf16, partition-aligned, with
           accum_out producing the per-head row sums l for free.
      tr : P[32g:32g+16, 128j:..] -> PT [128, 16] bf16 via PE
           transposes (identity replicated at partition bases 32g),
           then DVE tensor_scalar_mul x0.25 casts PT to... bf16 PT
           scaled by 1/4 so the later fp8-free path stays in range
           (exp(max ~6)=376 > fp8e4 max normal 240 was the nan source
           when PT was fp8; with bf16 PT the x0.25 simply keeps l and
           t consistent: host divides l by 4).
      T  : col-group g accumulates t for c-block g (512 cols) over 16
           s-chunks into its own PSUM bank; out[32g+h, j]=t[h,512g+j].
  * DMA schedule: 13 input stream DMAs in expected arrival order,
    ring-balanced, fine pieces early and where consumption trails;
    output as two small SWDGE (gpsimd) bf16 DMAs so no HW-DGE
    sem-slot-recycle wait ever stacks on a RAW wait.

Outputs per core: tt [128, 513] bf16 (4 partition groups of 16 heads x
512 c-cols, plus P-sums l in the last column of each group).  Host
merge: sum partials over cores, normalize by l/4, apply Wv per head,
then Wo + bo.

Sync-wait note: this backend disables DynamicDMA, so every HW-DGE DMA
lowers to a pseudo-direct DMA that supports at most ONE semaphore wait
("Too many sync wait commands" in walrus codegen otherwise).  The
program is therefore structured so no DMACopy ever needs two waits:
  - every streamed tile is a fresh buffer (unique pool tag, no reuse)
    so stream DMAs carry no WAR/WAW waits;
  - the program issues exactly 8 DMAs total (the 8 HW-DGE semaphore
    slots are assigned globally round-robin across both rings), so no
    DMA ever carries a slot-recycle wait on top of a RAW wait;
  - the output DMA's producers are all on the scalar engine;
  - the end-of-context Drain gets a sem wait for every proc the SP
    engine hasn't directly observed, so an epilogue of single-dep SP
    nops makes SP observe each DMA and each engine's last instruction.
"""

import sys

for _p in ("/root/.axon_site/_ro/trn_rl_repo", "/opt/trn_rl_repo"):
    if _p not in sys.path:
        sys.path.append(_p)

import numpy as np
import ml_dtypes

import concourse.bass as bass
import concourse.tile as tile
from concourse import mybir
from concourse.bass_utils import run_bass_kernel_spmd
from concourse.tile_rust import add_dep_helper

NCORES = 8
S_FULL = 16384
C = 2048           # input feature dim (both x1 and x2)
H = 16             # heads
J = 128            # head dim (K_DIM == V_DIM == 128)
HJ = H * J         # 2048
ODIM = 512
S_LOC = S_FULL // NCORES   # 2048 sequence rows per core

BF = mybir.dt.bfloat16
F32 = mybir.dt.float32
F8 = mybir.dt.float8e4
INV_SQRT_K = 1.0 / float(np.sqrt(128.0))
RSCALE = 64.0      # host multiplies R by this; Exp activation divides

NB = 512                    # PSUM bank free-dim (f32 columns)
CH = C // 128               # 16 chunks of 128 along any 2048 dim
NG = 4                      # column-tile groups

_F8_NP = ml_dtypes.float8_e4m3fn


PSCALE = 1.0    # PT is bf16 (no fp8 range concern); l and t consistent as-is


def _build_program() -> bass.Bass:
    nc = bass.Bass()
    # x2t/x2n are packed partition-major on the host ([p, chunk, col]) so a
    # multi-chunk stream DMA folds to ONE contiguous descriptor per partition.
    t_in = {
        "rsb": nc.dram_tensor("rsb", [J, CH, H], BF, kind="ExternalInput"),
        "x2t": nc.dram_tensor("x2t", [J, CH, S_LOC], F8, kind="ExternalInput"),
        "x2n": nc.dram_tensor("x2n", [J, CH, C], F8, kind="ExternalInput"),
    }
    t_out = {
        "tt": nc.dram_tensor("tt", [J, NB + 1], BF, kind="ExternalOutput"),
    }

    rsb_d = t_in["rsb"][:, :, :]
    x2t_v = t_in["x2t"][:, :, :]
    x2n_v = t_in["x2n"][:, :, :]
    tt_out = t_out["tt"][:, :]

    # Stream DMAs: (ring, first_chunk, n_chunks) in expected ARRIVAL order
    # (rings drain FIFO and run concurrently; PSUM accumulation is order-free,
    # so the matmul program order follows this schedule).  Input DMAs carry no
    # RAW wait (fresh buffers), so sem-slot recycling beyond 8 DMAs costs each
    # at most its one allowed wait.  Ring bytes are balanced; early pieces are
    # small to start compute sooner.
    PT_SCHED = [(0, 0, 1), (1, 1, 3), (0, 4, 3), (1, 7, 3),
                (0, 10, 3), (1, 13, 3)]
    PN_SCHED = [(0, 0, 1), (0, 1, 2), (1, 3, 3), (0, 6, 3),
                (1, 9, 3), (0, 12, 3), (1, 15, 1)]

    with tile.TileContext(nc) as tc:
        with (
            tc.tile_pool(name="singles", bufs=1) as singles,
            tc.tile_pool(name="sa", bufs=1) as sa,
            tc.tile_pool(name="sb", bufs=1) as sbp,
            tc.tile_pool(name="psbig", bufs=8, space="PSUM") as psbig,
        ):
            ep_targets = []     # representative instrs for the tail funnel
            stream_dmas = []    # input-stream DMAs: funneled EARLY (overlapped)

            # ---- issue every stream DMA up front (both rings) --------------
            Rsb = singles.tile([J, CH, H], BF)
            stream_dmas.append(nc.scalar.dma_start(out=Rsb, in_=rsb_d))

            s_tiles = []        # (first_cc, n_cc, tile)
            for ci, (ring, a, k) in enumerate(PT_SCHED):
                eng = nc.sync if ring == 0 else nc.scalar
                xt = sa.tile([128, k, S_LOC], F8, tag=f"sa_{ci}")
                stream_dmas.append(eng.dma_start(out=xt, in_=x2t_v[:, a : a + k, :]))
                s_tiles.append((a, k, xt))
            t_tiles = []
            for ci, (ring, a, k) in enumerate(PN_SCHED):
                eng = nc.sync if ring == 0 else nc.scalar
                xn = sbp.tile([128, k, C], F8, tag=f"sb_{ci}")
                stream_dmas.append(eng.dma_start(out=xn, in_=x2n_v[:, a : a + k, :]))
                t_tiles.append((a, k, xn))

            # Early drain-funnel for stream DMAs: sync nops that fire as each
            # piece lands (hidden under the stream); the end-of-context Drain
            # itself cannot carry a wait per DMA.
            for t in stream_dmas:
                n = nc.sync.nop(nofuse=True, hint="dep")
                add_dep_helper(n.ins, t.ins, reason="drain-funnel-early")



            # ---- phase S: scores[h, 512g+j] = sum_c R[c, h] x2t[c, 512g+j] -
            # col-group g owns s-block g.  One PSUM bank per group: four
            # interleaved accumulation groups sharing a bank corrupt
            # has_written state (stale-PSUM inf observed on HW).
            ps_s = [
                psbig.tile([J, NB], F32, tag="big", name=f"ps_s{g}")
                for g in range(NG)
            ]
            n_done = 0
            for pi, (a, k, xt) in enumerate(s_tiles):
                for gg in range(k):
                    cc = a + gg
                    n_done += 1
                    for g in range(NG):
                        nc.tensor.matmul(
                            ps_s[g][32 * g : 32 * g + H, :],
                            lhsT=Rsb[:, cc, :],
                            rhs=xt[:, gg, g * NB : (g + 1) * NB],
                            start=(n_done == 1),
                            stop=(n_done == CH),
                            tile_position=(0, 32 * g),
                        )


            # ---- exp (scale un-does RSCALE; |logit| < ~6) ------------------
            Psb = singles.tile([J, 4, 128], BF)   # [h(pad), s_hi, s_lo]
            lacc = singles.tile([J, 1], F32)
            i_act = None
            for g in range(NG):
                # accum_out gives the per-head row sums (l) for free -- the
                # partial sums per s-block land in the same partition group.
                i_act = nc.scalar.activation(
                    out=Psb[32 * g : 32 * g + H, :, :],
                    in_=ps_s[g][32 * g : 32 * g + H, :],
                    func=mybir.ActivationFunctionType.Exp,
                    scale=1.0 / RSCALE,
                    accum_out=lacc[32 * g : 32 * g + H, :],
                )

            # ---- transpose P -> PT [128, 16 schunk, 32(h pad)] bf16 --------
            # DVE blockwise 32x32 transposes straight from the exp output
            # (SBUF->SBUF, off the PE queue).  One instruction per (group g,
            # partition quarter m) covers 4 blocks via the strided [4, 32]
            # free pattern: chunk sc = 4g + j4, s = 128*sc + 32m + i.  Rows
            # 32g+16:32g+32 of Psb are unwritten pad; they land in PT columns
            # 16:32, which the T matmuls never read.
            PT = singles.tile([J, CH, 32], BF)
            for g in range(NG):
                for m in range(4):
                    nc.vector.transpose(
                        out=PT[32 * m : 32 * m + 32, 4 * g : 4 * g + 4, :],
                        in_=Psb[32 * g : 32 * g + 32, :, 32 * m : 32 * m + 32],
                    )

            # l columns of the output: copied early (deps ready at exp time),
            # hidden under the T-phase stream.
            tt_sb = singles.tile([J, NB + 1], BF)
            i_copies = []
            for g in range(NG):
                i_copies.append(
                    nc.vector.tensor_copy(
                        out=tt_sb[32 * g : 32 * g + H, NB : NB + 1],
                        in_=lacc[32 * g : 32 * g + H, :],
                    )
                )

            # ---- phase T: t[h, 512g+j] = sum_s PT[s, h] x2n[s, 512g+j] -----
            ps_t = [
                psbig.tile([J, NB], F32, tag="big", name=f"ps_t{g}")
                for g in range(NG)
            ]
            i_pe = None
            n_done = 0
            for a, k, xn in t_tiles:
                for gg in range(k):
                    sc = a + gg
                    n_done += 1
                    for g in range(NG):
                        i_pe = nc.tensor.matmul(
                            ps_t[g][32 * g : 32 * g + H, :],
                            lhsT=PT[:, sc, :H],
                            rhs=xn[:, gg, g * NB : (g + 1) * NB],
                            start=(n_done == 1),
                            stop=(n_done == CH),
                            tile_position=(0, 32 * g),
                        )
            # all output copies on DVE (idle at this point) so each output DMA
            # carries exactly one RAW wait; two SWDGE DMAs let the first
            # half's issue/transfer/receipt overlap the second half's copies.
            for g in range(NG):
                i_copies.append(
                    nc.vector.tensor_copy(
                        out=tt_sb[32 * g : 32 * g + H, :NB],
                        in_=ps_t[g][32 * g : 32 * g + H, :],
                    )
                )
            i_out = nc.gpsimd.dma_start(out=tt_out[:48, :], in_=tt_sb[:48, :])
            i_out2 = nc.gpsimd.dma_start(out=tt_out[64:112, :], in_=tt_sb[64:112, :])

            # ---- drain-funnel epilogue (see sync-wait note above): one nop
            # per un-observed proc -- each engine's LAST instruction plus the
            # output DMAs (stream DMAs were funneled early).
            ep_targets += [i_act, i_copies[-1], i_pe, i_out, i_out2]
            for t in ep_targets:
                n = nc.sync.nop(nofuse=True, hint="dep")
                add_dep_helper(n.ins, t.ins, reason="drain-funnel")

    return nc


_NC_CACHE = None


def _get_nc() -> bass.Bass:
    global _NC_CACHE
    if _NC_CACHE is None:
        _NC_CACHE = _build_program()
    return _NC_CACHE


def _prep_in_maps(x1, x2, Wq, Wk):
    x1 = np.asarray(x1, np.float32)
    x2 = np.asarray(x2, np.float32)
    Wq = np.asarray(Wq, np.float32)
    Wk = np.asarray(Wk, np.float32)

    # R[c, h] = sum_j Wk[h*128+j, c] q[h*128+j] / sqrt(128),  scaled by RSCALE
    q = (Wq @ x1) * (INV_SQRT_K * RSCALE)                       # [2048]
    R = np.einsum("hj,hjc->ch", q.reshape(H, J), Wk.reshape(H, J, C))
    rsb = np.ascontiguousarray(
        R.reshape(CH, 128, H).transpose(1, 0, 2)
    ).astype(ml_dtypes.bfloat16)                                # [128, 16, 16]

    in_maps = []
    for c in range(NCORES):
        shard = x2[c * S_LOC : (c + 1) * S_LOC]                 # [2048, 2048]
        # packed partition-major: x2t[p, cc, s] = shard.T[cc*128+p, s],
        #                         x2n[p, sc, c] = shard[sc*128+p, c]
        x2t_c = np.ascontiguousarray(
            shard.T.reshape(CH, 128, S_LOC).transpose(1, 0, 2)
        ).astype(_F8_NP)
        x2n_c = np.ascontiguousarray(
            shard.reshape(CH, 128, C).transpose(1, 0, 2)
        ).astype(_F8_NP)
        in_maps.append({"rsb": rsb, "x2t": x2t_c, "x2n": x2n_c})
    return in_maps


def _merge(results, Wv, Wo, bo):
    Wv = np.asarray(Wv, np.float32)
    Wo = np.asarray(Wo, np.float32)
    bo = np.asarray(bo, np.float32)
    t_tot = np.zeros((H, C), np.float64)
    l_tot = np.zeros(H, np.float64)
    for r in results:
        tt = r["tt"].astype(np.float64)                         # [128, 513]
        for g in range(NG):
            t_tot[:, g * NB : (g + 1) * NB] += tt[32 * g : 32 * g + H, :NB]
            l_tot += tt[32 * g : 32 * g + H, NB] * PSCALE
    tn = t_tot / l_tot[:, None]                                 # [16, 2048]
    u = np.einsum("hc,hjc->hj", tn, Wv.astype(np.float64).reshape(H, J, C))
    out = u.reshape(HJ) @ Wo.T.astype(np.float64) + bo.astype(np.float64)
    return out.astype(np.float32).reshape(1, ODIM)


def kernel(x1, x2, Wq, Wk, Wv, Wo, bo):
    nc = _get_nc()
    in_maps = _prep_in_maps(x1, x2, Wq, Wk)
    res = run_bass_kernel_spmd(nc, in_maps, list(range(NCORES)))
    return _merge(res.results, Wv, Wo, bo)


def run_traced(x1, x2, Wq, Wk, Wv, Wo, bo, **trace_kwargs):
    """Like kernel() but returns (output, BassKernelResults) with NTFF trace."""
    nc = _get_nc()
    in_maps = _prep_in_maps(x1, x2, Wq, Wk)
    res = run_bass_kernel_spmd(
        nc, in_maps, list(range(NCORES)), trace=True, **trace_kwargs
    )
    return _merge(res.results, Wv, Wo, bo), res


# revision 78
# speedup vs baseline: 1.4115x; 1.1193x over previous
"""Trainium2 Bass kernel for nn_CrossAttention_14207751815513.

Single-query cross-attention:
    q = x1 @ Wq.T                 (one query per head)
    k = x2 @ Wk.T ; v = x2 @ Wv.T
    attn_h = softmax(q_h . k_h / sqrt(128))
    out = concat_h(attn_h @ v_h) @ Wo.T + bo

Because there is exactly ONE query, the K and V projections collapse
algebraically (associativity):
    scores_h = x2 @ r_h,  r_h = Wk_h.T q_h / sqrt(128)   -- no k materialization
    out_h    = Wv_h @ (x2.T p_h) / l_h                   -- no v materialization
with p = exp(scores) (logits are small, |s| < ~6, so no max subtraction
is needed) and l_h = sum_s p_h[s].

Sharding: the sequence dim (16384) is split across the 8 NeuronCores
(2048 rows each).  Every quantity that is O(1) in the sequence length
(q, R = [r_1..r_16], the per-head Wv matvec, Wo + bias) lives in the
host-side shard-prep / gather-merge glue; the O(S*C) work runs on
device.

v2 changes vs the bf16 baseline (78us -> ~44us):
  * x2 streams (both layouts) are fp8e4 (e4m3): halves HBM->SBUF DMA
    bytes.  R (scaled x64 on the host into e4m3/bf16-friendly range;
    Exp applies scale=1/64) and PT stay bf16 -- the PE accepts mixed
    bf16-stationary x fp8-moving, and this keeps rel-err ~1.26e-2.
  * 4-way PE column tiling: all the big matmuls have only 16 output
    rows, so four run concurrently in distinct 32-column groups of the
    128x128 array (tile_position (0, 32g); one PSUM bank per group --
    interleaved accumulation groups sharing a bank corrupt has_written
    state).  Effective moving-operand ingest ~4 cols/cycle.
      S  : col-group g accumulates scores for s-block g (512 cols)
           over 16 c-chunks; all four into their own PSUM banks.
      exp: 4 ACT ops [16@32g, 512] -> P bf16, partition-aligned, with
           accum_out producing the per-head row sums l for free.
      tr : P[32g:32g+16, 128j:..] -> PT [128, 16] bf16 via PE
           transposes (identity replicated at partition bases 32g),
           then DVE tensor_scalar_mul x0.25 casts PT to... bf16 PT
           scaled by 1/4 so the later fp8-free path stays in range
           (exp(max ~6)=376 > fp8e4 max normal 240 was the nan source
           when PT was fp8; with bf16 PT the x0.25 simply keeps l and
           t consistent: host divides l by 4).
      T  : col-group g accumulates t for c-block g (512 cols) over 16
           s-chunks into its own PSUM bank; out[32g+h, j]=t[h,512g+j].
  * DMA schedule: 13 input stream DMAs in expected arrival order,
    ring-balanced, fine pieces early and where consumption trails;
    output as two small SWDGE (gpsimd) bf16 DMAs so no HW-DGE
    sem-slot-recycle wait ever stacks on a RAW wait.

Outputs per core: tt [128, 513] bf16 (4 partition groups of 16 heads x
512 c-cols, plus P-sums l in the last column of each group).  Host
merge: sum partials over cores, normalize by l/4, apply Wv per head,
then Wo + bo.

Sync-wait note: this backend disables DynamicDMA, so every HW-DGE DMA
lowers to a pseudo-direct DMA that supports at most ONE semaphore wait
("Too many sync wait commands" in walrus codegen otherwise).  The
program is therefore structured so no DMACopy ever needs two waits:
  - every streamed tile is a fresh buffer (unique pool tag, no reuse)
    so stream DMAs carry no WAR/WAW waits;
  - the program issues exactly 8 DMAs total (the 8 HW-DGE semaphore
    slots are assigned globally round-robin across both rings), so no
    DMA ever carries a slot-recycle wait on top of a RAW wait;
  - the output DMA's producers are all on the scalar engine;
  - the end-of-context Drain gets a sem wait for every proc the SP
    engine hasn't directly observed, so an epilogue of single-dep SP
    nops makes SP observe each DMA and each engine's last instruction.
"""

import sys

for _p in ("/root/.axon_site/_ro/trn_rl_repo", "/opt/trn_rl_repo"):
    if _p not in sys.path:
        sys.path.append(_p)

import numpy as np
import ml_dtypes

import concourse.bass as bass
import concourse.tile as tile
from concourse import mybir
from concourse.bass_utils import run_bass_kernel_spmd
from concourse.tile_rust import add_dep_helper

NCORES = 8
S_FULL = 16384
C = 2048           # input feature dim (both x1 and x2)
H = 16             # heads
J = 128            # head dim (K_DIM == V_DIM == 128)
HJ = H * J         # 2048
ODIM = 512
S_LOC = S_FULL // NCORES   # 2048 sequence rows per core

BF = mybir.dt.bfloat16
F32 = mybir.dt.float32
F8 = mybir.dt.float8e4
INV_SQRT_K = 1.0 / float(np.sqrt(128.0))
RSCALE = 64.0      # host multiplies R by this; Exp activation divides

NB = 512                    # PSUM bank free-dim (f32 columns)
CH = C // 128               # 16 chunks of 128 along any 2048 dim
NG = 4                      # column-tile groups

_F8_NP = ml_dtypes.float8_e4m3fn


PSCALE = 1.0    # PT is bf16 (no fp8 range concern); l and t consistent as-is


def _build_program() -> bass.Bass:
    nc = bass.Bass()
    # x2t/x2n are packed partition-major on the host ([p, chunk, col]) so a
    # multi-chunk stream DMA folds to ONE contiguous descriptor per partition.
    t_in = {
        "rsb": nc.dram_tensor("rsb", [J, CH, H], BF, kind="ExternalInput"),
        "x2t": nc.dram_tensor("x2t", [J, CH, S_LOC], F8, kind="ExternalInput"),
        "x2n": nc.dram_tensor("x2n", [J, CH, C], F8, kind="ExternalInput"),
    }
    t_out = {
        "tt": nc.dram_tensor("tt", [J, NB + 1], BF, kind="ExternalOutput"),
    }

    rsb_d = t_in["rsb"][:, :, :]
    x2t_v = t_in["x2t"][:, :, :]
    x2n_v = t_in["x2n"][:, :, :]
    tt_out = t_out["tt"][:, :]

    # Stream DMAs: (ring, first_chunk, n_chunks) in expected ARRIVAL order
    # (rings drain FIFO and run concurrently; PSUM accumulation is order-free,
    # so the matmul program order follows this schedule).  Input DMAs carry no
    # RAW wait (fresh buffers), so sem-slot recycling beyond 8 DMAs costs each
    # at most its one allowed wait.  Ring bytes are balanced; early pieces are
    # small to start compute sooner.
    PT_SCHED = [(0, 0, 1), (1, 1, 3), (0, 4, 3), (1, 7, 3),
                (0, 10, 3), (1, 13, 3)]
    PN_SCHED = [(0, 0, 1), (0, 1, 2), (1, 3, 3), (0, 6, 3),
                (1, 9, 3), (0, 12, 3), (1, 15, 1)]

    with tile.TileContext(nc) as tc:
        with (
            tc.tile_pool(name="singles", bufs=1) as singles,
            tc.tile_pool(name="sa", bufs=1) as sa,
            tc.tile_pool(name="sb", bufs=1) as sbp,
            tc.tile_pool(name="psbig", bufs=8, space="PSUM") as psbig,
        ):
            ep_targets = []     # representative instrs for the tail funnel
            stream_dmas = []    # input-stream DMAs: funneled EARLY (overlapped)

            # ---- issue every stream DMA up front (both rings) --------------
            Rsb = singles.tile([J, CH, H], BF)
            stream_dmas.append(nc.scalar.dma_start(out=Rsb, in_=rsb_d))

            s_tiles = []        # (first_cc, n_cc, tile)
            for ci, (ring, a, k) in enumerate(PT_SCHED):
                eng = nc.sync if ring == 0 else nc.scalar
                xt = sa.tile([128, k, S_LOC], F8, tag=f"sa_{ci}")
                stream_dmas.append(eng.dma_start(out=xt, in_=x2t_v[:, a : a + k, :]))
                s_tiles.append((a, k, xt))
            t_tiles = []
            for ci, (ring, a, k) in enumerate(PN_SCHED):
                eng = nc.sync if ring == 0 else nc.scalar
                xn = sbp.tile([128, k, C], F8, tag=f"sb_{ci}")
                stream_dmas.append(eng.dma_start(out=xn, in_=x2n_v[:, a : a + k, :]))
                t_tiles.append((a, k, xn))

            # Early drain-funnel for stream DMAs: sync nops that fire as each
            # piece lands (hidden under the stream); the end-of-context Drain
            # itself cannot carry a wait per DMA.
            for t in stream_dmas:
                n = nc.sync.nop(nofuse=True, hint="dep")
                add_dep_helper(n.ins, t.ins, reason="drain-funnel-early")



            # ---- phase S: scores[h, 512g+j] = sum_c R[c, h] x2t[c, 512g+j] -
            # col-group g owns s-block g.  One PSUM bank per group: four
            # interleaved accumulation groups sharing a bank corrupt
            # has_written state (stale-PSUM inf observed on HW).
            ps_s = [
                psbig.tile([J, NB], F32, tag="big", name=f"ps_s{g}")
                for g in range(NG)
            ]
            n_done = 0
            for pi, (a, k, xt) in enumerate(s_tiles):
                for gg in range(k):
                    cc = a + gg
                    n_done += 1
                    for g in range(NG):
                        nc.tensor.matmul(
                            ps_s[g][32 * g : 32 * g + H, :],
                            lhsT=Rsb[:, cc, :],
                            rhs=xt[:, gg, g * NB : (g + 1) * NB],
                            start=(n_done == 1),
                            stop=(n_done == CH),
                            tile_position=(0, 32 * g),
                        )


            # ---- exp (scale un-does RSCALE; |logit| < ~6) ------------------
            Psb = singles.tile([J, 4, 128], BF)   # [h(pad), s_hi, s_lo]
            lacc = singles.tile([J, 1], F32)
            i_act = None
            for g in range(NG):
                # accum_out gives the per-head row sums (l) for free -- the
                # partial sums per s-block land in the same partition group.
                i_act = nc.scalar.activation(
                    out=Psb[32 * g : 32 * g + H, :, :],
                    in_=ps_s[g][32 * g : 32 * g + H, :],
                    func=mybir.ActivationFunctionType.Exp,
                    scale=1.0 / RSCALE,
                    accum_out=lacc[32 * g : 32 * g + H, :],
                )

            # ---- transpose P -> PT [128, 16 schunk, 32(h pad)] bf16 --------
            # DVE blockwise 32x32 transposes straight from the exp output
            # (SBUF->SBUF, off the PE queue).  One instruction per (group g,
            # partition quarter m) covers 4 blocks via the strided [4, 32]
            # free pattern: chunk sc = 4g + j4, s = 128*sc + 32m + i.  Rows
            # 32g+16:32g+32 of Psb are unwritten pad; they land in PT columns
            # 16:32, which the T matmuls never read.
            PT = singles.tile([J, CH, 32], BF)
            for g in range(NG):
                for m in range(4):
                    nc.vector.transpose(
                        out=PT[32 * m : 32 * m + 32, 4 * g : 4 * g + 4, :],
                        in_=Psb[32 * g : 32 * g + 32, :, 32 * m : 32 * m + 32],
                    )

            # l columns of the output: copied early (deps ready at exp time),
            # hidden under the T-phase stream.  Groups 0,1 go via scalar and
            # 2,3 via vector so the two output DMAs each carry one RAW wait
            # and the final PSUM->SBUF casts run on two engines concurrently.
            tt_sb = singles.tile([J, NB + 1], BF)
            i_copies = []
            for g in range(NG):
                cp = nc.scalar.copy if g < 2 else nc.vector.tensor_copy
                i_copies.append(
                    cp(
                        out=tt_sb[32 * g : 32 * g + H, NB : NB + 1],
                        in_=lacc[32 * g : 32 * g + H, :],
                    )
                )

            # ---- phase T: t[h, 512g+j] = sum_s PT[s, h] x2n[s, 512g+j] -----
            ps_t = [
                psbig.tile([J, NB], F32, tag="big", name=f"ps_t{g}")
                for g in range(NG)
            ]
            i_pe = None
            n_done = 0
            for a, k, xn in t_tiles:
                for gg in range(k):
                    sc = a + gg
                    n_done += 1
                    for g in range(NG):
                        i_pe = nc.tensor.matmul(
                            ps_t[g][32 * g : 32 * g + H, :],
                            lhsT=PT[:, sc, :H],
                            rhs=xn[:, gg, g * NB : (g + 1) * NB],
                            start=(n_done == 1),
                            stop=(n_done == CH),
                            tile_position=(0, 32 * g),
                        )
            for g in range(NG):
                cp = nc.scalar.copy if g < 2 else nc.vector.tensor_copy
                i_copies.append(
                    cp(
                        out=tt_sb[32 * g : 32 * g + H, :NB],
                        in_=ps_t[g][32 * g : 32 * g + H, :],
                    )
                )
            i_out = nc.gpsimd.dma_start(out=tt_out[:48, :], in_=tt_sb[:48, :])
            i_out2 = nc.gpsimd.dma_start(out=tt_out[64:112, :], in_=tt_sb[64:112, :])

            # ---- drain-funnel epilogue (see sync-wait note above): one nop
            # per un-observed proc -- each engine's LAST instruction plus the
            # output DMAs (stream DMAs were funneled early).
            ep_targets += [i_copies[5], i_copies[-1], i_pe, i_out, i_out2]
            for t in ep_targets:
                n = nc.sync.nop(nofuse=True, hint="dep")
                add_dep_helper(n.ins, t.ins, reason="drain-funnel")

    return nc


_NC_CACHE = None


def _get_nc() -> bass.Bass:
    global _NC_CACHE
    if _NC_CACHE is None:
        _NC_CACHE = _build_program()
    return _NC_CACHE


def _prep_in_maps(x1, x2, Wq, Wk):
    x1 = np.asarray(x1, np.float32)
    x2 = np.asarray(x2, np.float32)
    Wq = np.asarray(Wq, np.float32)
    Wk = np.asarray(Wk, np.float32)

    # R[c, h] = sum_j Wk[h*128+j, c] q[h*128+j] / sqrt(128),  scaled by RSCALE
    q = (Wq @ x1) * (INV_SQRT_K * RSCALE)                       # [2048]
    R = np.einsum("hj,hjc->ch", q.reshape(H, J), Wk.reshape(H, J, C))
    rsb = np.ascontiguousarray(
        R.reshape(CH, 128, H).transpose(1, 0, 2)
    ).astype(ml_dtypes.bfloat16)                                # [128, 16, 16]

    in_maps = []
    for c in range(NCORES):
        shard = x2[c * S_LOC : (c + 1) * S_LOC]                 # [2048, 2048]
        # packed partition-major: x2t[p, cc, s] = shard.T[cc*128+p, s],
        #                         x2n[p, sc, c] = shard[sc*128+p, c]
        x2t_c = np.ascontiguousarray(
            shard.T.reshape(CH, 128, S_LOC).transpose(1, 0, 2)
        ).astype(_F8_NP)
        x2n_c = np.ascontiguousarray(
            shard.reshape(CH, 128, C).transpose(1, 0, 2)
        ).astype(_F8_NP)
        in_maps.append({"rsb": rsb, "x2t": x2t_c, "x2n": x2n_c})
    return in_maps


def _merge(results, Wv, Wo, bo):
    Wv = np.asarray(Wv, np.float32)
    Wo = np.asarray(Wo, np.float32)
    bo = np.asarray(bo, np.float32)
    t_tot = np.zeros((H, C), np.float64)
    l_tot = np.zeros(H, np.float64)
    for r in results:
        tt = r["tt"].astype(np.float64)                         # [128, 513]
        for g in range(NG):
            t_tot[:, g * NB : (g + 1) * NB] += tt[32 * g : 32 * g + H, :NB]
            l_tot += tt[32 * g : 32 * g + H, NB] * PSCALE
    tn = t_tot / l_tot[:, None]                                 # [16, 2048]
    u = np.einsum("hc,hjc->hj", tn, Wv.astype(np.float64).reshape(H, J, C))
    out = u.reshape(HJ) @ Wo.T.astype(np.float64) + bo.astype(np.float64)
    return out.astype(np.float32).reshape(1, ODIM)


def kernel(x1, x2, Wq, Wk, Wv, Wo, bo):
    nc = _get_nc()
    in_maps = _prep_in_maps(x1, x2, Wq, Wk)
    res = run_bass_kernel_spmd(nc, in_maps, list(range(NCORES)))
    return _merge(res.results, Wv, Wo, bo)


def run_traced(x1, x2, Wq, Wk, Wv, Wo, bo, **trace_kwargs):
    """Like kernel() but returns (output, BassKernelResults) with NTFF trace."""
    nc = _get_nc()
    in_maps = _prep_in_maps(x1, x2, Wq, Wk)
    res = run_bass_kernel_spmd(
        nc, in_maps, list(range(NCORES)), trace=True, **trace_kwargs
    )
    return _merge(res.results, Wv, Wo, bo), res
